# revision 1
# baseline (speedup 1.0000x reference)
"""Delta-accumulation GRU kernel for Trainium2 (8 NeuronCores, no
collectives; data-parallel over batch, 64 rows/core).

Gate pre-activations live in PSUM across all 64 steps:
    S_t = S_{t-1} + d_{t-1} @ W     where d = h_t - h_{t-1}
so steps 3..64 stream only delta matmuls (no bias rows), and the critical
recurrence chain is   z -> zc=sigmoid(-S_z) -> d = zc*(n-h) -> transpose(d)
(h' = h + d is off the critical path).  d and v'=n-h are bf16: the delta
multiply runs in DVE 2x mode and the transposes/assembly copies are
half-width; this also *improves* accuracy (h's carry matches the bf16
deltas the PSUM accumulation sees).  The delta multiply is split into
512-column halves so the first transpose batch overlaps the second half.

Column groups (concurrent PE streams, measured ~87% overlap):
    group0 (cols 0-63 -> psum parts 0-63):   r0 r1 gin0 z1
    group1 (cols 64-127 -> psum parts 64-127): ghn0 ghn1 gin1 z0
so every n-path input (r, ghn, gin) lands a chunk-slot before the final
z chunks, leaving only the short zc -> d chain after the last matmul.

PSUM (8 banks, all persistent; has_written bits are per-partition, so the
staging transposes at parts 0-63 of pC's banks don't disturb the ghn
accumulation at parts 64-127):
    pA [128,1024]: r (parts 0-63)
    pB [128,1024]: gin0 | z1 (parts 0-63)
    pC [128,1024]: ghn (parts 64-127) + transpose staging (parts 0-63)
    pD [128,1024]: gin1 | z0 (parts 64-127)

Verified on HW: rel_err 0.00835 vs the fp32 reference (gate 2e-2);
~9-10us/step measured by drift-cancelling paired step-count differentials
(v1 baseline was 16.8us/step).
"""

import numpy as np
import ml_dtypes

import concourse.bass as bass
import concourse.bacc as bacc
import concourse.mybir as mybir
import concourse.tile as tile
from concourse.bass_utils import run_bass_kernel_spmd
from concourse.masks import make_identity

BF16 = mybir.dt.bfloat16
F32 = mybir.dt.float32
AF = mybir.ActivationFunctionType
ALU = mybir.AluOpType

B, D, T = 512, 1024, 64
NCORES = 8
BL = B // NCORES
CTX = 3072
NK = D // 128
NKC = CTX // 128
CH = 512

_CACHE = {}
TRACE = False
TRACE_KW = {}
LAST_RESULT = [None]
LAST_IN_MAPS = [None]


def _build_nc(n_steps=T, act_copies=False, gpsimd_h=False, chunked_dd=False,
              bf16_d=True):
    nc = bacc.Bacc("TRN2")

    ctxT_h = nc.declare_dram_parameter("ctxT", [CTX, BL], BF16, isOutput=False)
    wctx_h = nc.declare_dram_parameter("wctx", [CTX, D], BF16, isOutput=False)
    whh_h = nc.declare_dram_parameter("whh", [D, 3 * D], BF16, isOutput=False)
    wall_h = nc.declare_dram_parameter("wall", [D, 4 * D], BF16, isOutput=False)
    bctx_h = nc.declare_dram_parameter("bctx", [1, D], BF16, isOutput=False)
    bias1_h = nc.declare_dram_parameter("bias1", [1, 3 * D], BF16, isOutput=False)
    gin1_h = nc.declare_dram_parameter("gin1", [1, D], F32, isOutput=False)
    biasM_h = nc.declare_dram_parameter("biasM", [1, 4 * D], BF16, isOutput=False)
    out_h = nc.declare_dram_parameter("out", [T, BL, D], F32, isOutput=True)

    with tile.TileContext(nc) as tc:
        with (
            tc.tile_pool(name="wres", bufs=1) as wres,
            tc.tile_pool(name="wstream", bufs=4) as wstream,
            tc.tile_pool(name="consts", bufs=1) as consts,
            tc.tile_pool(name="state", bufs=2) as state,
            tc.tile_pool(name="work", bufs=1) as work,
            tc.tile_pool(name="psum", bufs=1, space="PSUM") as psum,
        ):
            ctxT_sb = consts.tile([128, NKC, BL], BF16)
            nc.sync.dma_start(
                out=ctxT_sb, in_=ctxT_h[:].rearrange("(ko p) b -> p ko b", p=128)
            )
            # split the big weight loads so they spread across DMA queues
            whh_sb = wres.tile([128, NK, 3 * D], BF16, tag="whh")
            whh_t = whh_h[:].rearrange("(ko p) n -> p ko n", p=128)
            for q in range(4):
                nc.sync.dma_start(
                    out=whh_sb[:, 2 * q : 2 * q + 2, :],
                    in_=whh_t[:, 2 * q : 2 * q + 2, :],
                )
            wall_sb = wres.tile([128, NK, 4 * D], BF16, tag="wall")
            wall_t = wall_h[:].rearrange("(ko p) n -> p ko n", p=128)
            for q in range(4):
                nc.sync.dma_start(
                    out=wall_sb[:, 2 * q : 2 * q + 2, :],
                    in_=wall_t[:, 2 * q : 2 * q + 2, :],
                )
            wctx_t = wctx_h[:].rearrange("(ko p) n -> p ko n", p=128)
            bctx_sb = consts.tile([1, D], BF16)
            nc.sync.dma_start(out=bctx_sb, in_=bctx_h[:])
            bias1_sb = consts.tile([1, 3 * D], BF16)
            nc.sync.dma_start(out=bias1_sb, in_=bias1_h[:])
            biasM_sb = consts.tile([1, 4 * D], BF16)
            nc.sync.dma_start(out=biasM_sb, in_=biasM_h[:])
            gin1_bc = consts.tile([BL, D], F32)
            g1 = gin1_h[:]
            g1_bc = bass.AP(tensor=g1.tensor, offset=g1.offset, ap=[[0, BL], [1, D]])
            nc.gpsimd.dma_start(out=gin1_bc, in_=g1_bc)
            ones_sb = consts.tile([1, BL], BF16)
            nc.vector.memset(ones_sb, 1.0)
            ident_sb = consts.tile([BL, BL], F32)
            make_identity(nc, ident_sb)
            ident_bf = consts.tile([BL, BL], BF16)
            make_identity(nc, ident_bf)

            # persistent PSUM accumulators
            pA = psum.tile([128, D], F32, tag="pA")
            pB = psum.tile([128, D], F32, tag="pB")
            pC = psum.tile([128, D], F32, tag="pC")
            pD = psum.tile([128, D], F32, tag="pD")
            r_ap = pA[0:64, :]
            gin0_ap = pB[0:64, 0:CH]
            z1_ap = pB[0:64, CH:D]
            ghn_ap = pC[64:128, :]
            gin1_ap = pD[64:128, 0:CH]
            z0_ap = pD[64:128, CH:D]
            # transpose staging at parts 0-63 of pC's banks (gates live at
            # parts 64-127 there; has_written bits are per-partition so the
            # start=True staging writes don't disturb the accumulation)
            stage_ap = pC[0:64, :]
            stage_bf = pC[0:64, :].bitcast(BF16)  # [64, 2048] bf16 view

            def transpose_half(src_sb, half, bf=False):
                """stage transposes of src[:, half*512 : half*512+512]."""
                stage = stage_bf if bf else stage_ap
                ident = ident_bf if bf else ident_sb
                for j in range(half * NK, (half + 1) * NK):
                    nc.tensor.transpose(
                        stage[:, j * 64 : (j + 1) * 64],
                        src_sb[:, j * 64 : (j + 1) * 64],
                        ident,
                    )

            def assemble_xT(i, bf=False):
                """staged pieces -> xT [128, NK, 64] bf16 (copies on ScalarE
                to keep DVE off the critical chain)."""
                xT = state.tile(
                    [128, NK, BL], BF16, tag="xT", bufs=2, name=f"xT_{i}"
                )
                # piece 2k -> xT rows 0-63 of k-tile; piece 2k+1 -> rows 64-127
                stage = stage_bf if bf else stage_ap
                stg = (
                    stage[:, 0 : 16 * 64]
                    .rearrange("p (k two j) -> p k two j", two=2, j=64)
                )
                cpy = nc.scalar.copy if act_copies else nc.vector.tensor_copy
                cpy(xT[0:64, :, :], stg[:, :, 0, :])
                cpy(xT[64:128, :, :], stg[:, :, 1, :])
                return xT

            def transpose_to(src_sb, i, bf=False):
                transpose_half(src_sb, 0, bf)
                transpose_half(src_sb, 1, bf)
                return assemble_xT(i, bf)

            # (psum_ap, wcol, tile_position) chunk specs per step kind
            def chunks_steady(gin_c, ghn_c):
                g0 = [(r_ap[:, 0:CH], 0, (0, 0)),
                      (r_ap[:, CH:D], CH, (0, 0)),
                      (gin0_ap, gin_c, (0, 0)),
                      (z1_ap, 3 * CH, (0, 0))]
                g1 = [(ghn_ap[:, 0:CH], ghn_c, (0, 64)),
                      (ghn_ap[:, CH:D], ghn_c + CH, (0, 64)),
                      (gin1_ap, gin_c + CH, (0, 64)),
                      (z0_ap, 2 * CH, (0, 64))]
                return list(zip(g0, g1))

            def chunks_step1(ghn_c):
                g0 = [(r_ap[:, 0:CH], 0, (0, 0)),
                      (r_ap[:, CH:D], CH, (0, 0)),
                      (z1_ap, 3 * CH, (0, 0))]
                g1 = [(ghn_ap[:, 0:CH], ghn_c, (0, 64)),
                      (ghn_ap[:, CH:D], ghn_c + CH, (0, 64)),
                      (z0_ap, 2 * CH, (0, 64))]
                return list(zip(g0, g1))

            def mm_phase(pairs, xT, w_sb, bias_sb, full):
                if full:
                    for pair in pairs:
                        for pap, wcol, tp in pair:
                            nc.tensor.matmul(
                                pap, ones_sb[0:1, :],
                                bias_sb[0:1, wcol : wcol + CH],
                                start=True, stop=False, tile_position=tp,
                            )
                for pair in pairs:
                    for k in range(NK):
                        for pap, wcol, tp in pair:
                            nc.tensor.matmul(
                                pap, xT[:, k, :],
                                w_sb[:, k, wcol : wcol + CH],
                                start=False, stop=(k == NK - 1),
                                tile_position=tp,
                            )

            def tail(s, hprev, step1, last):
                i = nc.next_id()
                rs = work.tile([BL, D], BF16, tag="rs", name=f"rs_{i}")
                nc.scalar.activation(rs, r_ap, AF.Sigmoid)
                tt = work.tile([BL, D], F32, tag="tt", name=f"tt_{i}")
                nc.vector.tensor_mul(tt, rs, ghn_ap)
                uu = work.tile([BL, D], F32, tag="uu", name=f"uu_{i}")
                if step1:
                    nc.vector.tensor_add(uu, tt, gin1_bc)
                else:
                    nc.vector.tensor_add(uu[:, 0:CH], tt[:, 0:CH], gin0_ap)
                    nc.vector.tensor_add(uu[:, CH:D], tt[:, CH:D], gin1_ap)
                nn_sb = work.tile([BL, D], F32, tag="nn", name=f"nn_{i}")
                nc.scalar.activation(nn_sb, uu, AF.Tanh)
                # bf16 vp makes the critical dd-mul all-2-byte -> DVE 2x mode
                vp = work.tile([BL, D], BF16 if bf16_d else F32, tag="vp",
                               name=f"vp_{i}")
                nc.vector.tensor_sub(vp, nn_sb, hprev)  # v' = n - h
                # z0 covers z-gate cols 0-511 (wall cols 1024-1535), z1 the
                # rest; zc = sigmoid(-S_z) = 1-z, so d = zc * (n-h) = h'-h
                zc = work.tile([BL, D], BF16, tag="zc", name=f"zc_{i}")
                nc.scalar.activation(zc[:, 0:CH], z0_ap, AF.Sigmoid, scale=-1.0)
                nc.scalar.activation(zc[:, CH:D], z1_ap, AF.Sigmoid, scale=-1.0)
                dd = work.tile([BL, D], BF16 if bf16_d else F32, tag="dd",
                               name=f"dd_{i}")
                hnew = state.tile([BL, D], F32, bufs=3, tag="h", name=f"h_{i}")
                hadd = nc.gpsimd.tensor_add if gpsimd_h else nc.vector.tensor_add
                if step1 or last:
                    nc.vector.tensor_mul(dd, zc, vp)
                    hadd(hnew, hprev, dd)
                    nc.sync.dma_start(out=out_h[s], in_=hnew)
                    if last:
                        return hnew, None
                    xT = transpose_to(hnew, i)  # step 2 needs h1T
                    return hnew, xT
                # steady: critical chain is dd -> transposes -> copies; the
                # h' update and output DMA happen off-path
                if chunked_dd:
                    nc.vector.tensor_mul(dd[:, 0:CH], zc[:, 0:CH], vp[:, 0:CH])
                    transpose_half(dd, 0, bf16_d)
                    nc.vector.tensor_mul(dd[:, CH:D], zc[:, CH:D], vp[:, CH:D])
                    transpose_half(dd, 1, bf16_d)
                    xT = assemble_xT(i, bf16_d)
                else:
                    # split mul: dd half 0 (gated on zc0 only) unblocks its
                    # transpose batch ~0.7us before zc1's half completes
                    nc.vector.tensor_mul(dd[:, 0:CH], zc[:, 0:CH], vp[:, 0:CH])
                    nc.vector.tensor_mul(dd[:, CH:D], zc[:, CH:D], vp[:, CH:D])
                    transpose_half(dd, 0, bf16_d)
                    transpose_half(dd, 1, bf16_d)
                    xT = assemble_xT(i, bf16_d)
                hadd(hnew, hprev, dd)
                nc.sync.dma_start(out=out_h[s], in_=hnew)
                return hnew, xT

            # ---- h0 (into pA parts 0-63, before step 1 overwrites) ----
            ph0 = pA[0:64, :]
            for c in range(2):
                nc.tensor.matmul(
                    ph0[:, c * CH : (c + 1) * CH], ones_sb[0:1, :],
                    bctx_sb[0:1, c * CH : (c + 1) * CH],
                    start=True, stop=False, tile_position=(0, 0),
                )
            for kc in range(NKC):
                wk = wstream.tile([128, D], BF16, tag="wctxk", name=f"wk_{kc}")
                nc.gpsimd.dma_start(out=wk, in_=wctx_t[:, kc, :])
                for c in range(2):
                    nc.tensor.matmul(
                        ph0[:, c * CH : (c + 1) * CH], ctxT_sb[:, kc, :],
                        wk[:, c * CH : (c + 1) * CH],
                        start=False, stop=(kc == NKC - 1), tile_position=(0, 0),
                    )
            h0_sb = state.tile([BL, D], F32, tag="h", bufs=3)
            nc.vector.tensor_copy(h0_sb, ph0)
            hT0 = transpose_to(h0_sb, 0)

            # ---- step 1: full write, W_hh ----
            mm_phase(chunks_step1(2 * D), hT0, whh_sb, bias1_sb, full=True)
            hprev, xT = tail(0, h0_sb, step1=True, last=(n_steps == 1))

            # ---- step 2: full write, W_all, stationary h1T ----
            if n_steps >= 2:
                mm_phase(chunks_steady(2 * D, 3 * D), xT, wall_sb, biasM_sb,
                         full=True)
                hprev, xT = tail(1, hprev, step1=False, last=(n_steps == 2))

            # ---- steps 3..n: accumulate deltas ----
            # (n_steps > T is a timing-only mode: out index wraps)
            for s in range(2, n_steps):
                mm_phase(chunks_steady(2 * D, 3 * D), xT, wall_sb, biasM_sb,
                         full=False)
                hprev, xT = tail(s % T, hprev, step1=False,
                                 last=(s == n_steps - 1))

    nc.finalize()
    return nc


def kernel(world_state, goal, W_ctx, b_ctx, start_token, W_ih, b_ih, W_hh, b_hh):
    bf16 = ml_dtypes.bfloat16
    ws = np.asarray(world_state, dtype=np.float32)
    gl = np.asarray(goal, dtype=np.float32)
    W_ctx = np.asarray(W_ctx, dtype=np.float32)
    b_ctx = np.asarray(b_ctx, dtype=np.float32)
    start_token = np.asarray(start_token, dtype=np.float32)
    W_ih = np.asarray(W_ih, dtype=np.float32)
    b_ih = np.asarray(b_ih, dtype=np.float32)
    W_hh = np.asarray(W_hh, dtype=np.float32)
    b_hh = np.asarray(b_hh, dtype=np.float32)

    if "nc" not in _CACHE:
        _CACHE["nc"] = _build_nc()
    nc = _CACHE["nc"]

    ctxT = np.ascontiguousarray(np.concatenate([ws, gl], axis=1).T)
    ctxT_bf = ctxT.astype(bf16)
    wctx_bf = np.ascontiguousarray(W_ctx).astype(bf16)
    whh_bf = np.ascontiguousarray(W_hh).astype(bf16)
    wall_bf = np.ascontiguousarray(
        np.concatenate(
            [W_ih[:, : 2 * D] + W_hh[:, : 2 * D], W_ih[:, 2 * D :], W_hh[:, 2 * D :]],
            axis=1,
        )
    ).astype(bf16)
    gi1 = start_token @ W_ih + b_ih
    bias1 = np.ascontiguousarray(
        np.concatenate([gi1[: 2 * D] + b_hh[: 2 * D], b_hh[2 * D :]])
    ).astype(bf16)[None]
    gin1 = np.ascontiguousarray(gi1[2 * D :].astype(np.float32))[None]
    biasM = np.ascontiguousarray(
        np.concatenate([b_ih[: 2 * D] + b_hh[: 2 * D], b_ih[2 * D :], b_hh[2 * D :]])
    ).astype(bf16)[None]
    bctx = np.ascontiguousarray(b_ctx).astype(bf16)[None]

    shared = dict(
        wctx=wctx_bf, whh=whh_bf, wall=wall_bf, bctx=bctx,
        bias1=bias1, gin1=gin1, biasM=biasM,
    )
    in_maps = [
        {**shared, "ctxT": np.ascontiguousarray(ctxT_bf[:, i * BL : (i + 1) * BL])}
        for i in range(NCORES)
    ]

    LAST_IN_MAPS[0] = in_maps
    res = run_bass_kernel_spmd(
        nc, in_maps, core_ids=list(range(NCORES)), trace=TRACE, **TRACE_KW
    )
    LAST_RESULT[0] = res

    full = np.empty((B, T, D), dtype=np.float32)
    for i in range(NCORES):
        o = np.asarray(res.results[i]["out"])
        full[i * BL : (i + 1) * BL] = o.transpose(1, 0, 2)
    return full

